# revision 9
# baseline (speedup 1.0000x reference)
"""Trainium2 8-core kernel for 2-layer GAT (nn_DiGCN_65335042507185).

Design (v2):
  Nodes are partitioned across 8 cores by dst (12500/core). Per core, dst
  nodes are bin-packed by in-degree into 392 groups of 32 nodes; each group
  owns 5 edge-tiles of 128 slots (640 capacity). Four NEFFs per call:

    A  (lin, F=128): xs1 = x@W1 and attention preacts s1,d1 on device.
    B  (agg, relu):  layer-1 edge softmax + one-hot aggregation.
    B2 (lin, F=64):  xs2 = h@W2 and preacts s2,d2 on device.
    C  (agg):        layer-2 aggregation -> final embeddings.

  The agg NEFF streams host-gathered xs[src] rows (64 feats + ones col,
  bf16) plus per-slot preact/dstloc. On device: LeakyReLU+exp (softmax
  numerator), a 32-wide one-hot built at DVE 2x mode ([P,G,TC] layout with
  materialized iota), weight folded into the one-hot, 32-col TensorE
  matmuls with tile_position packing 4 groups per PSUM bank, and
  normalization Relu(z^-1 * agg) on ScalarE. Host does graph partitioning,
  slot layout, gathers (halo exchange surrogate), and resharding only.
"""
import sys
for _p in ("/opt/trn_rl_repo", "/root/.axon_site/_ro/trn_rl_repo"):
    if _p not in sys.path:
        sys.path.insert(0, _p)

import numpy as np
import ml_dtypes
from contextlib import ExitStack

import concourse.bass as bass
import concourse.bacc as bacc
import concourse.mybir as mybir
import concourse.tile as tile
from concourse.bass_utils import run_bass_kernel_spmd

P = 128
N = 100_000
NFEAT = 128
NHID = 64
NEG_SLOPE = 0.2
NCORES = 8
NSH = 12500                  # real nodes per core
G = 32                       # dst nodes per group (one-hot width)
TPG = 5                      # tiles per group (640 edge slots capacity)
NGRP = 392                   # groups per core
NODES_PAD = NGRP * G         # 12544 node slots per core
NT = NGRP * TPG              # 1960 tiles per core
NSLOT = NT * P               # 250880 edge slots per core
GPC = 28                     # groups per chunk
TC = GPC * TPG               # 140 tiles per chunk
NCHUNK = NGRP // GPC         # 14
NPS = NGRP // 4              # 98 psum tiles (4 groups each)
PPC = GPC // 4               # 7 psum tiles per chunk
CS = 65                      # stream cols: 64 feats + ones
AF = mybir.ActivationFunctionType
DT = mybir.dt
ALU = mybir.AluOpType
BF16 = ml_dtypes.bfloat16
FP8 = False
F8 = ml_dtypes.float8_e4m3

_CACHE = {}


# ---------------------------------------------------------------- device ----

def _build_lin(F):
    """xs = x@W plus preacts s,d. In: xT [F,NODES_PAD] bf16, W [F,64] bf16,
    WT [64,F] bf16, apair [64,2] bf16. Out: xs_sd [66, NODES_PAD] bf16
    (rows 0:64 = xs^T, 64:66 = s,d)."""
    nc = bacc.Bacc("TRN2", target_bir_lowering=False, debug=False,
                   num_devices=NCORES)
    xT = nc.dram_tensor("xT", [F, NODES_PAD], DT.bfloat16,
                        kind="ExternalInput").ap()
    w_h = nc.dram_tensor("w", [F, NHID], DT.bfloat16, kind="ExternalInput").ap()
    wT_h = nc.dram_tensor("wT", [NHID, F], DT.bfloat16, kind="ExternalInput").ap()
    ap_h = nc.dram_tensor("apair", [NHID, 2], DT.bfloat16, kind="ExternalInput").ap()
    out_h = nc.dram_tensor("xs_sd", [NHID + 2, NODES_PAD], DT.bfloat16,
                           kind="ExternalOutput").ap()
    NTILE = NODES_PAD // P          # 98
    CHT = 14                        # node tiles per input DMA chunk
    with tile.TileContext(nc) as tc, ExitStack() as ctx:
        cpool = ctx.enter_context(tc.tile_pool(name="consts", bufs=1))
        wcat = cpool.tile([F, NHID + 2], DT.bfloat16)
        nc.sync.dma_start(wcat[:, 0:NHID], w_h[:])
        wT = cpool.tile([NHID, F], DT.bfloat16)
        nc.sync.dma_start(wT[:], wT_h[:])
        apair = cpool.tile([NHID, 2], DT.bfloat16)
        nc.sync.dma_start(apair[:], ap_h[:])
        with tc.tile_pool(name="va", bufs=1, space="PSUM") as vpool:
            va_ps = vpool.tile([F, 2], DT.float32)
            nc.tensor.matmul(va_ps[:], lhsT=wT[:], rhs=apair[:],
                             start=True, stop=True)
            nc.vector.tensor_copy(wcat[:, NHID:NHID + 2], va_ps[:])

        stage = cpool.tile([NHID + 2, NODES_PAD], DT.bfloat16)
        xp = ctx.enter_context(tc.tile_pool(name="x", bufs=2))
        pp = ctx.enter_context(tc.tile_pool(name="ps", bufs=8, space="PSUM"))
        MMW = 2 * P                       # rhs cols per matmul
        for ci in range(NTILE // CHT):
            xt = xp.tile([F, CHT * P], DT.bfloat16, tag="xt")
            nc.sync.dma_start(xt[:], xT[:, ci * CHT * P:(ci + 1) * CHT * P])
            for k in range(CHT * P // MMW):
                c0 = ci * CHT * P + k * MMW
                ps = pp.tile([NHID + 2, MMW], DT.float32, tag="ps")
                nc.tensor.matmul(ps[:], lhsT=wcat[:],
                                 rhs=xt[:, k * MMW:(k + 1) * MMW],
                                 start=True, stop=True)
                if k % 2 == 0:
                    nc.vector.tensor_copy(stage[:, c0:c0 + MMW], ps[:])
                else:
                    nc.scalar.activation(stage[:, c0:c0 + MMW], ps[:], AF.Copy)
        nc.sync.dma_start(out_h[:], stage[:])
    nc.compile()
    return nc


def _build_agg(relu, f32_out):
    """One GAT aggregation layer over the packed edge stream."""
    nc = bacc.Bacc("TRN2", target_bir_lowering=False, debug=False,
                   num_devices=NCORES)
    sdt = DT.float8e4 if FP8 else DT.bfloat16
    feats = nc.dram_tensor("feats", [P, NT, CS], sdt,
                           kind="ExternalInput").ap()
    pre_h = nc.dram_tensor("pre", [P, NT], DT.bfloat16, kind="ExternalInput").ap()
    dst_h = nc.dram_tensor("dstloc", [P, NT], DT.bfloat16,
                           kind="ExternalInput").ap()
    iota_h = nc.dram_tensor("iota", [P, G, TC], DT.bfloat16,
                            kind="ExternalInput").ap()
    odt = DT.float32 if f32_out else DT.bfloat16
    out_h = nc.dram_tensor("out", [P, NPS, NHID], odt, kind="ExternalOutput").ap()

    with tile.TileContext(nc) as tc, ExitStack() as ctx:
        cpool = ctx.enter_context(tc.tile_pool(name="consts", bufs=1))
        iota = cpool.tile([P, G, TC], DT.bfloat16)
        nc.sync.dma_start(iota[:], iota_h[:])

        sp = ctx.enter_context(tc.tile_pool(name="stream", bufs=3))
        mp = ctx.enter_context(tc.tile_pool(name="meta", bufs=2))
        ep = ctx.enter_context(tc.tile_pool(name="edge", bufs=2))
        mwp = ctx.enter_context(tc.tile_pool(name="mw", bufs=3))
        op = ctx.enter_context(tc.tile_pool(name="out", bufs=2))
        zp = ctx.enter_context(tc.tile_pool(name="z", bufs=8))
        pp = ctx.enter_context(tc.tile_pool(name="ps", bufs=8, space="PSUM"))

        def _evac(p):
            pl, pci = p
            outsb = op.tile([P, PPC, NHID], odt, tag="outsb")
            for k, ps in enumerate(pl):
                zinv = zp.tile([P, 1], DT.float32, tag="zinv")
                nc.vector.reciprocal(zinv[:], ps[:, NHID:NHID + 1])
                nc.scalar.activation(outsb[:, k, :], ps[:, 0:NHID],
                                     AF.Relu if relu else AF.Copy,
                                     scale=zinv[:])
            nc.sync.dma_start(out_h[:, pci * PPC:(pci + 1) * PPC, :], outsb[:])

        pend = None
        for ci in range(NCHUNK):
            t0 = ci * TC
            S = sp.tile([P, TC, CS], sdt, tag="S")
            nc.sync.dma_start(S[:], feats[:, t0:t0 + TC, :])
            pre = mp.tile([P, TC], DT.bfloat16, tag="pre")
            nc.sync.dma_start(pre[:], pre_h[:, t0:t0 + TC])
            dstl = mp.tile([P, TC], DT.bfloat16, tag="dstl")
            nc.sync.dma_start(dstl[:], dst_h[:, t0:t0 + TC])

            lk = ep.tile([P, TC], DT.float32, tag="lk")
            nc.vector.tensor_scalar(out=lk[:], in0=pre[:], scalar1=NEG_SLOPE,
                                    scalar2=None, op0=ALU.mult)
            nc.vector.tensor_tensor(out=lk[:], in0=lk[:], in1=pre[:], op=ALU.max)
            w = ep.tile([P, TC], DT.bfloat16, tag="w")
            nc.scalar.activation(w[:], lk[:], AF.Exp)

            M = mwp.tile([P, G, TC], DT.bfloat16, tag="M")
            nc.vector.tensor_tensor(
                out=M[:], in0=dstl[:, None, :].broadcast_to([P, G, TC]),
                in1=iota[:], op=ALU.is_equal)
            Mw = mwp.tile([P, G, TC], DT.bfloat16, tag="Mw")
            nc.vector.tensor_tensor(
                out=Mw[:], in0=M[:], in1=w[:, None, :].broadcast_to([P, G, TC]),
                op=ALU.mult)

            if pend is not None:
                _evac(pend)
            ps_list = []
            for k in range(PPC):
                ps = pp.tile([P, CS], DT.float32, tag="ps")
                for j in range(4):
                    gl = k * 4 + j
                    tt = gl * TPG
                    for t in range(TPG):
                        nc.tensor.matmul(ps[G * j:G * (j + 1), :],
                                         lhsT=Mw[:, :, tt + t],
                                         rhs=S[:, tt + t, :],
                                         start=(t == 0), stop=(t == TPG - 1),
                                         tile_position=(0, G * j))
                ps_list.append(ps)
            pend = (ps_list, ci)
        _evac(pend)
    nc.compile()
    return nc


def _get(key, builder, *a):
    if key not in _CACHE:
        _CACHE[key] = builder(*a)
    return _CACHE[key]


# ------------------------------------------------------------------ host ----

def _bin_pack(deg):
    """LPT: assign NSH nodes to NGRP bins of exactly G slots, load<=TPG*P.
    Returns perm [NGRP*G] int32 (node id or -1 for pad)."""
    import heapq
    order = np.argsort(-deg, kind="stable")
    heap = [(0, g) for g in range(NGRP)]
    heapq.heapify(heap)
    bins = [[] for _ in range(NGRP)]
    spill = []
    for n in order:
        d = int(deg[n])
        load, g = heapq.heappop(heap)
        while len(bins[g]) >= G:
            load, g = heapq.heappop(heap)
        bins[g].append(n)
        nl = load + d
        if nl > TPG * P:
            raise RuntimeError(f"bin overflow {nl}")
        if len(bins[g]) < G:
            heapq.heappush(heap, (nl, g))
        else:
            spill.append((nl, g))
    perm = np.full(NGRP * G, -1, dtype=np.int64)
    for g, lst in enumerate(bins):
        perm[g * G:g * G + len(lst)] = lst
    return perm


def _prep_graph(edge_index):
    """Per-core slot layout. Returns list of dicts."""
    ei = np.asarray(edge_index)
    src = np.concatenate([ei[0], np.arange(N, dtype=ei.dtype)]).astype(np.int64)
    dst = np.concatenate([ei[1], np.arange(N, dtype=ei.dtype)]).astype(np.int64)
    owner = dst // NSH
    cores = []
    for c in range(NCORES):
        sel = owner == c
        s_c = src[sel]
        d_c = dst[sel] - c * NSH                     # local dst 0..12499
        deg = np.bincount(d_c, minlength=NSH)
        perm = _bin_pack(deg)                        # [12544] node or -1
        # node -> (group, j)
        slot_of_node = np.full(NSH, -1, dtype=np.int64)
        valid = perm >= 0
        slot_of_node[perm[valid]] = np.nonzero(valid)[0]
        key = slot_of_node[d_c]                      # g*32+j per edge
        order = np.argsort(key, kind="stable")
        s_c, d_c, key = s_c[order], d_c[order], key[order]
        grp = key // G
        # position within group: running index
        gstart = np.searchsorted(grp, np.arange(NGRP))
        cnt = np.diff(np.append(gstart, len(grp)))
        if cnt.max() > TPG * P:
            raise RuntimeError(f"group overflow {cnt.max()}")
        pos = np.arange(len(grp)) - gstart[grp]
        slot = grp * (TPG * P) + pos                 # linear slot in [0, NSLOT)
        slot_src = np.zeros(NSLOT, dtype=np.int64)
        slot_dst_g = np.zeros(NSLOT, dtype=np.int64) # global dst per slot
        dstloc = np.zeros(NSLOT, dtype=np.float32)
        wkill = np.full(NSLOT, True)                 # pad slots
        slot_src[slot] = s_c
        slot_dst_g[slot] = d_c + c * NSH
        dstloc[slot] = key % G
        wkill[slot] = False
        cores.append(dict(slot_src=slot_src, slot_dst=slot_dst_g,
                          dstloc=dstloc.astype(BF16), wkill=wkill, perm=perm))
    return cores


def _make_iota():
    i = np.arange(G, dtype=np.float32)[None, :, None]
    return np.broadcast_to(i, (P, G, TC)).astype(BF16).copy()


def _feats_stream(table66, core):
    """table66 [N,65] (col 64 = 1.0). -> [P, NT, CS] stream dtype."""
    flat = table66[core["slot_src"]]                 # [NSLOT, 65]
    flat[core["wkill"], 64] = 0                      # ones col 0 on pad slots
    return np.ascontiguousarray(
        flat.reshape(NT, P, CS).transpose(1, 0, 2))


def _meta_streams(pre_f32, core):
    pre = pre_f32.copy()
    pre[core["wkill"]] = -30000.0
    pre = pre.astype(BF16).reshape(NT, P).T.copy()
    dstl = core["dstloc"].reshape(NT, P).T.copy()
    return pre, dstl


def _run_lin(nc_lin, xT_list, W, a_src, a_dst):
    Wb = np.ascontiguousarray(W, dtype=np.float32).astype(BF16)
    WTb = np.ascontiguousarray(W.T, dtype=np.float32).astype(BF16)
    ap = np.stack([a_src, a_dst], axis=1).astype(np.float32).astype(BF16)
    in_maps = [{"xT": xT_list[c], "w": Wb, "wT": WTb, "apair": ap}
               for c in range(NCORES)]
    res = run_bass_kernel_spmd(nc_lin, in_maps, core_ids=list(range(NCORES)))
    # assemble global tables: xs [N,64] bf16 (from cols 0:NSH), s,d [N] f32
    xs = np.empty((N, NHID + 2), dtype=np.float32)
    for c in range(NCORES):
        xs[c * NSH:(c + 1) * NSH] = \
            res.results[c]["xs_sd"][:, :NSH].T.astype(np.float32)
    return xs[:, 0:NHID], xs[:, NHID], xs[:, NHID + 1]


def _run_agg(nc_agg, cores, xs, s, d, iota):
    table66 = np.empty((N, CS), dtype=np.float32)
    table66[:, 0:NHID] = xs
    table66[:, NHID] = 1.0
    table66 = table66.astype(F8 if FP8 else BF16)
    in_maps = []
    for core in cores:
        pre = s[core["slot_src"]] + d[core["slot_dst"]]
        pre_st, dst_st = _meta_streams(pre, core)
        in_maps.append({"feats": _feats_stream(table66, core),
                        "pre": pre_st, "dstloc": dst_st, "iota": iota})
    res = run_bass_kernel_spmd(nc_agg, in_maps, core_ids=list(range(NCORES)))
    # out [P, NPS, 64] -> rows r = pstile*128+p = g*32+j -> node perm[g*32+j]
    full = np.empty((N, NHID), dtype=np.float32)
    for c, core in enumerate(cores):
        o = res.results[c]["out"]                   # [P, NPS, 64]
        rows = o.transpose(1, 0, 2).reshape(NODES_PAD, NHID).astype(np.float32)
        valid = core["perm"] >= 0
        full[c * NSH + core["perm"][valid]] = rows[valid]
    return full


def kernel(x, W1, att_src1, att_dst1, W2, att_src2, att_dst2, edge_index):
    x = np.asarray(x, dtype=np.float32)
    W1 = np.asarray(W1, dtype=np.float32)
    W2 = np.asarray(W2, dtype=np.float32)
    a_s1 = np.asarray(att_src1, dtype=np.float32)
    a_d1 = np.asarray(att_dst1, dtype=np.float32)
    a_s2 = np.asarray(att_src2, dtype=np.float32)
    a_d2 = np.asarray(att_dst2, dtype=np.float32)

    cores = _prep_graph(edge_index)
    iota = _make_iota()

    ncA = _get(("lin", NFEAT), _build_lin, NFEAT)
    ncB2 = _get(("lin", NHID), _build_lin, NHID)
    ncB = _get(("agg", True), _build_agg, True, False)
    ncC = _get(("agg", False), _build_agg, False, True)

    # layer 1
    xb = x.astype(BF16)
    xT_list = []
    for c in range(NCORES):
        xt = np.zeros((NFEAT, NODES_PAD), dtype=BF16)
        xt[:, :NSH] = xb[c * NSH:(c + 1) * NSH].T
        xT_list.append(xt)
    xs1, s1, d1 = _run_lin(ncA, xT_list, W1, a_s1, a_d1)
    h = _run_agg(ncB, cores, xs1, s1, d1, iota)

    # layer 2
    hb = h.astype(BF16)
    hT_list = []
    for c in range(NCORES):
        ht = np.zeros((NHID, NODES_PAD), dtype=BF16)
        ht[:, :NSH] = hb[c * NSH:(c + 1) * NSH].T
        hT_list.append(ht)
    xs2, s2, d2 = _run_lin(ncB2, hT_list, W2, a_s2, a_d2)
    out = _run_agg(ncC, cores, xs2, s2, d2, iota)
    return out.astype(np.float32)


# revision 10
# speedup vs baseline: 1.0091x; 1.0091x over previous
"""Trainium2 8-core kernel for 2-layer GAT (nn_DiGCN_65335042507185).

Design (v2):
  Nodes are partitioned across 8 cores by dst (12500/core). Per core, dst
  nodes are bin-packed by in-degree into 392 groups of 32 nodes; each group
  owns 5 edge-tiles of 128 slots (640 capacity). Four NEFFs per call:

    A  (lin, F=128): xs1 = x@W1 and attention preacts s1,d1 on device.
    B  (agg, relu):  layer-1 edge softmax + one-hot aggregation.
    B2 (lin, F=64):  xs2 = h@W2 and preacts s2,d2 on device.
    C  (agg):        layer-2 aggregation -> final embeddings.

  The agg NEFF streams host-gathered xs[src] rows (64 feats + ones col,
  bf16) plus per-slot preact/dstloc. On device: LeakyReLU+exp (softmax
  numerator), a 32-wide one-hot built at DVE 2x mode ([P,G,TC] layout with
  materialized iota), weight folded into the one-hot, 32-col TensorE
  matmuls with tile_position packing 4 groups per PSUM bank, and
  normalization Relu(z^-1 * agg) on ScalarE. Host does graph partitioning,
  slot layout, gathers (halo exchange surrogate), and resharding only.
"""
import sys
for _p in ("/opt/trn_rl_repo", "/root/.axon_site/_ro/trn_rl_repo"):
    if _p not in sys.path:
        sys.path.insert(0, _p)

import numpy as np
import ml_dtypes
from contextlib import ExitStack

import concourse.bass as bass
import concourse.bacc as bacc
import concourse.mybir as mybir
import concourse.tile as tile
from concourse.bass_utils import run_bass_kernel_spmd

P = 128
N = 100_000
NFEAT = 128
NHID = 64
NEG_SLOPE = 0.2
NCORES = 8
NSH = 12500                  # real nodes per core
G = 32                       # dst nodes per group (one-hot width)
TPG = 5                      # tiles per group (640 edge slots capacity)
NGRP = 392                   # groups per core
NODES_PAD = NGRP * G         # 12544 node slots per core
NT = NGRP * TPG              # 1960 tiles per core
NSLOT = NT * P               # 250880 edge slots per core
GPC = 28                     # groups per chunk
TC = GPC * TPG               # 140 tiles per chunk
NCHUNK = NGRP // GPC         # 14
NPS = NGRP // 4              # 98 psum tiles (4 groups each)
PPC = GPC // 4               # 7 psum tiles per chunk
CS = 65                      # stream cols: 64 feats + ones
AF = mybir.ActivationFunctionType
DT = mybir.dt
ALU = mybir.AluOpType
BF16 = ml_dtypes.bfloat16
FP8 = False
F8 = ml_dtypes.float8_e4m3

_CACHE = {}


# ---------------------------------------------------------------- device ----

def _build_lin(F):
    """xs = x@W plus preacts s,d. In: xT [F,NODES_PAD] bf16, W [F,64] bf16,
    WT [64,F] bf16, apair [64,2] bf16. Out: xs_sd [66, NODES_PAD] bf16
    (rows 0:64 = xs^T, 64:66 = s,d)."""
    nc = bacc.Bacc("TRN2", target_bir_lowering=False, debug=False,
                   num_devices=NCORES)
    xT = nc.dram_tensor("xT", [F, NODES_PAD], DT.bfloat16,
                        kind="ExternalInput").ap()
    w_h = nc.dram_tensor("w", [F, NHID], DT.bfloat16, kind="ExternalInput").ap()
    wT_h = nc.dram_tensor("wT", [NHID, F], DT.bfloat16, kind="ExternalInput").ap()
    ap_h = nc.dram_tensor("apair", [NHID, 2], DT.bfloat16, kind="ExternalInput").ap()
    out_h = nc.dram_tensor("xs_sd", [NHID + 2, NODES_PAD], DT.bfloat16,
                           kind="ExternalOutput").ap()
    NTILE = NODES_PAD // P          # 98
    CHT = 14                        # node tiles per input DMA chunk
    with tile.TileContext(nc) as tc, ExitStack() as ctx:
        cpool = ctx.enter_context(tc.tile_pool(name="consts", bufs=1))
        wcat = cpool.tile([F, NHID + 2], DT.bfloat16)
        nc.sync.dma_start(wcat[:, 0:NHID], w_h[:])
        wT = cpool.tile([NHID, F], DT.bfloat16)
        nc.sync.dma_start(wT[:], wT_h[:])
        apair = cpool.tile([NHID, 2], DT.bfloat16)
        nc.sync.dma_start(apair[:], ap_h[:])
        with tc.tile_pool(name="va", bufs=1, space="PSUM") as vpool:
            va_ps = vpool.tile([F, 2], DT.float32)
            nc.tensor.matmul(va_ps[:], lhsT=wT[:], rhs=apair[:],
                             start=True, stop=True)
            nc.vector.tensor_copy(wcat[:, NHID:NHID + 2], va_ps[:])

        xp = ctx.enter_context(tc.tile_pool(name="x", bufs=3))
        stp = ctx.enter_context(tc.tile_pool(name="stage", bufs=3))
        pp = ctx.enter_context(tc.tile_pool(name="ps", bufs=8, space="PSUM"))
        MMW = 2 * P                       # rhs cols per matmul
        for ci in range(NTILE // CHT):
            xt = xp.tile([F, CHT * P], DT.bfloat16, tag="xt")
            nc.sync.dma_start(xt[:], xT[:, ci * CHT * P:(ci + 1) * CHT * P])
            stage = stp.tile([NHID + 2, CHT * P], DT.bfloat16, tag="stage")
            for k in range(CHT * P // MMW):
                c0 = k * MMW
                ps = pp.tile([NHID + 2, MMW], DT.float32, tag="ps")
                nc.tensor.matmul(ps[:], lhsT=wcat[:],
                                 rhs=xt[:, k * MMW:(k + 1) * MMW],
                                 start=True, stop=True)
                if k % 2 == 0:
                    nc.vector.tensor_copy(stage[:, c0:c0 + MMW], ps[:])
                else:
                    nc.scalar.activation(stage[:, c0:c0 + MMW], ps[:], AF.Copy)
            nc.sync.dma_start(out_h[:, ci * CHT * P:(ci + 1) * CHT * P], stage[:])
    nc.compile()
    return nc


def _build_agg(relu, f32_out):
    """One GAT aggregation layer over the packed edge stream."""
    nc = bacc.Bacc("TRN2", target_bir_lowering=False, debug=False,
                   num_devices=NCORES)
    sdt = DT.float8e4 if FP8 else DT.bfloat16
    feats = nc.dram_tensor("feats", [P, NT, CS], sdt,
                           kind="ExternalInput").ap()
    pre_h = nc.dram_tensor("pre", [P, NT], DT.bfloat16, kind="ExternalInput").ap()
    dst_h = nc.dram_tensor("dstloc", [P, NT], DT.bfloat16,
                           kind="ExternalInput").ap()
    iota_h = nc.dram_tensor("iota", [P, G, TC], DT.bfloat16,
                            kind="ExternalInput").ap()
    odt = DT.float32 if f32_out else DT.bfloat16
    out_h = nc.dram_tensor("out", [P, NPS, NHID], odt, kind="ExternalOutput").ap()

    with tile.TileContext(nc) as tc, ExitStack() as ctx:
        cpool = ctx.enter_context(tc.tile_pool(name="consts", bufs=1))
        iota = cpool.tile([P, G, TC], DT.bfloat16)
        nc.sync.dma_start(iota[:], iota_h[:])

        sp = ctx.enter_context(tc.tile_pool(name="stream", bufs=3))
        mp = ctx.enter_context(tc.tile_pool(name="meta", bufs=2))
        ep = ctx.enter_context(tc.tile_pool(name="edge", bufs=2))
        mwp = ctx.enter_context(tc.tile_pool(name="mw", bufs=3))
        op = ctx.enter_context(tc.tile_pool(name="out", bufs=2))
        zp = ctx.enter_context(tc.tile_pool(name="z", bufs=8))
        pp = ctx.enter_context(tc.tile_pool(name="ps", bufs=8, space="PSUM"))

        def _evac(p):
            pl, pci = p
            outsb = op.tile([P, PPC, NHID], odt, tag="outsb")
            for k, ps in enumerate(pl):
                zinv = zp.tile([P, 1], DT.float32, tag="zinv")
                nc.vector.reciprocal(zinv[:], ps[:, NHID:NHID + 1])
                nc.scalar.activation(outsb[:, k, :], ps[:, 0:NHID],
                                     AF.Relu if relu else AF.Copy,
                                     scale=zinv[:])
            nc.sync.dma_start(out_h[:, pci * PPC:(pci + 1) * PPC, :], outsb[:])

        pend = None
        for ci in range(NCHUNK):
            t0 = ci * TC
            S = sp.tile([P, TC, CS], sdt, tag="S")
            nc.sync.dma_start(S[:], feats[:, t0:t0 + TC, :])
            pre = mp.tile([P, TC], DT.bfloat16, tag="pre")
            nc.sync.dma_start(pre[:], pre_h[:, t0:t0 + TC])
            dstl = mp.tile([P, TC], DT.bfloat16, tag="dstl")
            nc.sync.dma_start(dstl[:], dst_h[:, t0:t0 + TC])

            lk = ep.tile([P, TC], DT.float32, tag="lk")
            nc.vector.tensor_scalar(out=lk[:], in0=pre[:], scalar1=NEG_SLOPE,
                                    scalar2=None, op0=ALU.mult)
            nc.vector.tensor_tensor(out=lk[:], in0=lk[:], in1=pre[:], op=ALU.max)
            w = ep.tile([P, TC], DT.bfloat16, tag="w")
            nc.scalar.activation(w[:], lk[:], AF.Exp)

            M = mwp.tile([P, G, TC], DT.bfloat16, tag="M")
            nc.vector.tensor_tensor(
                out=M[:], in0=dstl[:, None, :].broadcast_to([P, G, TC]),
                in1=iota[:], op=ALU.is_equal)
            Mw = mwp.tile([P, G, TC], DT.bfloat16, tag="Mw")
            nc.vector.tensor_tensor(
                out=Mw[:], in0=M[:], in1=w[:, None, :].broadcast_to([P, G, TC]),
                op=ALU.mult)

            if pend is not None:
                _evac(pend)
            ps_list = []
            for k in range(PPC):
                ps = pp.tile([P, CS], DT.float32, tag="ps")
                for j in range(4):
                    gl = k * 4 + j
                    tt = gl * TPG
                    for t in range(TPG):
                        nc.tensor.matmul(ps[G * j:G * (j + 1), :],
                                         lhsT=Mw[:, :, tt + t],
                                         rhs=S[:, tt + t, :],
                                         start=(t == 0), stop=(t == TPG - 1),
                                         tile_position=(0, G * j))
                ps_list.append(ps)
            pend = (ps_list, ci)
        _evac(pend)
    nc.compile()
    return nc


def _get(key, builder, *a):
    if key not in _CACHE:
        _CACHE[key] = builder(*a)
    return _CACHE[key]


# ------------------------------------------------------------------ host ----

def _bin_pack(deg):
    """LPT: assign NSH nodes to NGRP bins of exactly G slots, load<=TPG*P.
    Returns perm [NGRP*G] int32 (node id or -1 for pad)."""
    import heapq
    order = np.argsort(-deg, kind="stable")
    heap = [(0, g) for g in range(NGRP)]
    heapq.heapify(heap)
    bins = [[] for _ in range(NGRP)]
    spill = []
    for n in order:
        d = int(deg[n])
        load, g = heapq.heappop(heap)
        while len(bins[g]) >= G:
            load, g = heapq.heappop(heap)
        bins[g].append(n)
        nl = load + d
        if nl > TPG * P:
            raise RuntimeError(f"bin overflow {nl}")
        if len(bins[g]) < G:
            heapq.heappush(heap, (nl, g))
        else:
            spill.append((nl, g))
    perm = np.full(NGRP * G, -1, dtype=np.int64)
    for g, lst in enumerate(bins):
        perm[g * G:g * G + len(lst)] = lst
    return perm


def _prep_graph(edge_index):
    """Per-core slot layout. Returns list of dicts."""
    ei = np.asarray(edge_index)
    src = np.concatenate([ei[0], np.arange(N, dtype=ei.dtype)]).astype(np.int64)
    dst = np.concatenate([ei[1], np.arange(N, dtype=ei.dtype)]).astype(np.int64)
    owner = dst // NSH
    cores = []
    for c in range(NCORES):
        sel = owner == c
        s_c = src[sel]
        d_c = dst[sel] - c * NSH                     # local dst 0..12499
        deg = np.bincount(d_c, minlength=NSH)
        perm = _bin_pack(deg)                        # [12544] node or -1
        # node -> (group, j)
        slot_of_node = np.full(NSH, -1, dtype=np.int64)
        valid = perm >= 0
        slot_of_node[perm[valid]] = np.nonzero(valid)[0]
        key = slot_of_node[d_c]                      # g*32+j per edge
        order = np.argsort(key, kind="stable")
        s_c, d_c, key = s_c[order], d_c[order], key[order]
        grp = key // G
        # position within group: running index
        gstart = np.searchsorted(grp, np.arange(NGRP))
        cnt = np.diff(np.append(gstart, len(grp)))
        if cnt.max() > TPG * P:
            raise RuntimeError(f"group overflow {cnt.max()}")
        pos = np.arange(len(grp)) - gstart[grp]
        slot = grp * (TPG * P) + pos                 # linear slot in [0, NSLOT)
        slot_src = np.zeros(NSLOT, dtype=np.int64)
        slot_dst_g = np.zeros(NSLOT, dtype=np.int64) # global dst per slot
        dstloc = np.zeros(NSLOT, dtype=np.float32)
        wkill = np.full(NSLOT, True)                 # pad slots
        slot_src[slot] = s_c
        slot_dst_g[slot] = d_c + c * NSH
        dstloc[slot] = key % G
        wkill[slot] = False
        cores.append(dict(slot_src=slot_src, slot_dst=slot_dst_g,
                          dstloc=dstloc.astype(BF16), wkill=wkill, perm=perm))
    return cores


def _make_iota():
    i = np.arange(G, dtype=np.float32)[None, :, None]
    return np.broadcast_to(i, (P, G, TC)).astype(BF16).copy()


def _feats_stream(table66, core):
    """table66 [N,65] (col 64 = 1.0). -> [P, NT, CS] stream dtype."""
    flat = table66[core["slot_src"]]                 # [NSLOT, 65]
    flat[core["wkill"], 64] = 0                      # ones col 0 on pad slots
    return np.ascontiguousarray(
        flat.reshape(NT, P, CS).transpose(1, 0, 2))


def _meta_streams(pre_f32, core):
    pre = pre_f32.copy()
    pre[core["wkill"]] = -30000.0
    pre = pre.astype(BF16).reshape(NT, P).T.copy()
    dstl = core["dstloc"].reshape(NT, P).T.copy()
    return pre, dstl


def _run_lin(nc_lin, xT_list, W, a_src, a_dst):
    Wb = np.ascontiguousarray(W, dtype=np.float32).astype(BF16)
    WTb = np.ascontiguousarray(W.T, dtype=np.float32).astype(BF16)
    ap = np.stack([a_src, a_dst], axis=1).astype(np.float32).astype(BF16)
    in_maps = [{"xT": xT_list[c], "w": Wb, "wT": WTb, "apair": ap}
               for c in range(NCORES)]
    res = run_bass_kernel_spmd(nc_lin, in_maps, core_ids=list(range(NCORES)))
    # assemble global tables: xs [N,64] bf16 (from cols 0:NSH), s,d [N] f32
    xs = np.empty((N, NHID + 2), dtype=np.float32)
    for c in range(NCORES):
        xs[c * NSH:(c + 1) * NSH] = \
            res.results[c]["xs_sd"][:, :NSH].T.astype(np.float32)
    return xs[:, 0:NHID], xs[:, NHID], xs[:, NHID + 1]


def _run_agg(nc_agg, cores, xs, s, d, iota):
    table66 = np.empty((N, CS), dtype=np.float32)
    table66[:, 0:NHID] = xs
    table66[:, NHID] = 1.0
    table66 = table66.astype(F8 if FP8 else BF16)
    in_maps = []
    for core in cores:
        pre = s[core["slot_src"]] + d[core["slot_dst"]]
        pre_st, dst_st = _meta_streams(pre, core)
        in_maps.append({"feats": _feats_stream(table66, core),
                        "pre": pre_st, "dstloc": dst_st, "iota": iota})
    res = run_bass_kernel_spmd(nc_agg, in_maps, core_ids=list(range(NCORES)))
    # out [P, NPS, 64] -> rows r = pstile*128+p = g*32+j -> node perm[g*32+j]
    full = np.empty((N, NHID), dtype=np.float32)
    for c, core in enumerate(cores):
        o = res.results[c]["out"]                   # [P, NPS, 64]
        rows = o.transpose(1, 0, 2).reshape(NODES_PAD, NHID).astype(np.float32)
        valid = core["perm"] >= 0
        full[c * NSH + core["perm"][valid]] = rows[valid]
    return full


def kernel(x, W1, att_src1, att_dst1, W2, att_src2, att_dst2, edge_index):
    x = np.asarray(x, dtype=np.float32)
    W1 = np.asarray(W1, dtype=np.float32)
    W2 = np.asarray(W2, dtype=np.float32)
    a_s1 = np.asarray(att_src1, dtype=np.float32)
    a_d1 = np.asarray(att_dst1, dtype=np.float32)
    a_s2 = np.asarray(att_src2, dtype=np.float32)
    a_d2 = np.asarray(att_dst2, dtype=np.float32)

    cores = _prep_graph(edge_index)
    iota = _make_iota()

    ncA = _get(("lin", NFEAT), _build_lin, NFEAT)
    ncB2 = _get(("lin", NHID), _build_lin, NHID)
    ncB = _get(("agg", True), _build_agg, True, False)
    ncC = _get(("agg", False), _build_agg, False, True)

    # layer 1
    xb = x.astype(BF16)
    xT_list = []
    for c in range(NCORES):
        xt = np.zeros((NFEAT, NODES_PAD), dtype=BF16)
        xt[:, :NSH] = xb[c * NSH:(c + 1) * NSH].T
        xT_list.append(xt)
    xs1, s1, d1 = _run_lin(ncA, xT_list, W1, a_s1, a_d1)
    h = _run_agg(ncB, cores, xs1, s1, d1, iota)

    # layer 2
    hb = h.astype(BF16)
    hT_list = []
    for c in range(NCORES):
        ht = np.zeros((NHID, NODES_PAD), dtype=BF16)
        ht[:, :NSH] = hb[c * NSH:(c + 1) * NSH].T
        hT_list.append(ht)
    xs2, s2, d2 = _run_lin(ncB2, hT_list, W2, a_s2, a_d2)
    out = _run_agg(ncC, cores, xs2, s2, d2, iota)
    return out.astype(np.float32)


# revision 11
# speedup vs baseline: 1.0250x; 1.0158x over previous
"""Trainium2 8-core kernel for 2-layer GAT (nn_DiGCN_65335042507185).

Design (v2):
  Nodes are partitioned across 8 cores by dst (12500/core). Per core, dst
  nodes are bin-packed by in-degree into 392 groups of 32 nodes; each group
  owns 5 edge-tiles of 128 slots (640 capacity). Four NEFFs per call:

    A  (lin, F=128): xs1 = x@W1 and attention preacts s1,d1 on device.
    B  (agg, relu):  layer-1 edge softmax + one-hot aggregation.
    B2 (lin, F=64):  xs2 = h@W2 and preacts s2,d2 on device.
    C  (agg):        layer-2 aggregation -> final embeddings.

  The agg NEFF streams host-gathered xs[src] rows (64 feats + ones col,
  bf16) plus per-slot preact/dstloc. On device: LeakyReLU+exp (softmax
  numerator), a 32-wide one-hot built at DVE 2x mode ([P,G,TC] layout with
  materialized iota), weight folded into the one-hot, 32-col TensorE
  matmuls with tile_position packing 4 groups per PSUM bank, and
  normalization Relu(z^-1 * agg) on ScalarE. Host does graph partitioning,
  slot layout, gathers (halo exchange surrogate), and resharding only.
"""
import sys
for _p in ("/opt/trn_rl_repo", "/root/.axon_site/_ro/trn_rl_repo"):
    if _p not in sys.path:
        sys.path.insert(0, _p)

import numpy as np
import ml_dtypes
from contextlib import ExitStack

import concourse.bass as bass
import concourse.bacc as bacc
import concourse.mybir as mybir
import concourse.tile as tile
from concourse.bass_utils import run_bass_kernel_spmd

P = 128
N = 100_000
NFEAT = 128
NHID = 64
NEG_SLOPE = 0.2
NCORES = 8
NSH = 12500                  # real nodes per core
G = 32                       # dst nodes per group (one-hot width)
TPG = 5                      # tiles per group (640 edge slots capacity)
NGRP = 392                   # groups per core
NODES_PAD = NGRP * G         # 12544 node slots per core
NT = NGRP * TPG              # 1960 tiles per core
NSLOT = NT * P               # 250880 edge slots per core
GPC = 56                     # groups per chunk
TC = GPC * TPG               # 140 tiles per chunk
NCHUNK = NGRP // GPC         # 14
NPS = NGRP // 4              # 98 psum tiles (4 groups each)
PPC = GPC // 4               # 7 psum tiles per chunk
CS = 65                      # stream cols: 64 feats + ones
AF = mybir.ActivationFunctionType
DT = mybir.dt
ALU = mybir.AluOpType
BF16 = ml_dtypes.bfloat16
FP8 = False
F8 = ml_dtypes.float8_e4m3

_CACHE = {}


# ---------------------------------------------------------------- device ----

def _build_lin(F):
    """xs = x@W plus preacts s,d. In: xT [F,NODES_PAD] bf16, W [F,64] bf16,
    WT [64,F] bf16, apair [64,2] bf16. Out: xs_sd [66, NODES_PAD] bf16
    (rows 0:64 = xs^T, 64:66 = s,d)."""
    nc = bacc.Bacc("TRN2", target_bir_lowering=False, debug=False,
                   num_devices=NCORES)
    xT = nc.dram_tensor("xT", [F, NODES_PAD], DT.bfloat16,
                        kind="ExternalInput").ap()
    w_h = nc.dram_tensor("w", [F, NHID], DT.bfloat16, kind="ExternalInput").ap()
    wT_h = nc.dram_tensor("wT", [NHID, F], DT.bfloat16, kind="ExternalInput").ap()
    ap_h = nc.dram_tensor("apair", [NHID, 2], DT.bfloat16, kind="ExternalInput").ap()
    out_h = nc.dram_tensor("xs_sd", [NHID + 2, NODES_PAD], DT.bfloat16,
                           kind="ExternalOutput").ap()
    NTILE = NODES_PAD // P          # 98
    CHT = 14                        # node tiles per input DMA chunk
    with tile.TileContext(nc) as tc, ExitStack() as ctx:
        cpool = ctx.enter_context(tc.tile_pool(name="consts", bufs=1))
        wcat = cpool.tile([F, NHID + 2], DT.bfloat16)
        nc.sync.dma_start(wcat[:, 0:NHID], w_h[:])
        wT = cpool.tile([NHID, F], DT.bfloat16)
        nc.sync.dma_start(wT[:], wT_h[:])
        apair = cpool.tile([NHID, 2], DT.bfloat16)
        nc.sync.dma_start(apair[:], ap_h[:])
        with tc.tile_pool(name="va", bufs=1, space="PSUM") as vpool:
            va_ps = vpool.tile([F, 2], DT.float32)
            nc.tensor.matmul(va_ps[:], lhsT=wT[:], rhs=apair[:],
                             start=True, stop=True)
            nc.vector.tensor_copy(wcat[:, NHID:NHID + 2], va_ps[:])

        xp = ctx.enter_context(tc.tile_pool(name="x", bufs=3))
        stp = ctx.enter_context(tc.tile_pool(name="stage", bufs=3))
        pp = ctx.enter_context(tc.tile_pool(name="ps", bufs=8, space="PSUM"))
        MMW = 2 * P                       # rhs cols per matmul
        for ci in range(NTILE // CHT):
            xt = xp.tile([F, CHT * P], DT.bfloat16, tag="xt")
            nc.sync.dma_start(xt[:], xT[:, ci * CHT * P:(ci + 1) * CHT * P])
            stage = stp.tile([NHID + 2, CHT * P], DT.bfloat16, tag="stage")
            for k in range(CHT * P // MMW):
                c0 = k * MMW
                ps = pp.tile([NHID + 2, MMW], DT.float32, tag="ps")
                nc.tensor.matmul(ps[:], lhsT=wcat[:],
                                 rhs=xt[:, k * MMW:(k + 1) * MMW],
                                 start=True, stop=True)
                if k % 2 == 0:
                    nc.vector.tensor_copy(stage[:, c0:c0 + MMW], ps[:])
                else:
                    nc.scalar.activation(stage[:, c0:c0 + MMW], ps[:], AF.Copy)
            nc.sync.dma_start(out_h[:, ci * CHT * P:(ci + 1) * CHT * P], stage[:])
    nc.compile()
    return nc


def _build_agg(relu, f32_out):
    """One GAT aggregation layer over the packed edge stream."""
    nc = bacc.Bacc("TRN2", target_bir_lowering=False, debug=False,
                   num_devices=NCORES)
    sdt = DT.float8e4 if FP8 else DT.bfloat16
    feats = nc.dram_tensor("feats", [P, NT, CS], sdt,
                           kind="ExternalInput").ap()
    pre_h = nc.dram_tensor("pre", [P, NT], DT.bfloat16, kind="ExternalInput").ap()
    dst_h = nc.dram_tensor("dstloc", [P, NT], DT.bfloat16,
                           kind="ExternalInput").ap()
    iota_h = nc.dram_tensor("iota", [P, G, TC], DT.bfloat16,
                            kind="ExternalInput").ap()
    odt = DT.float32 if f32_out else DT.bfloat16
    out_h = nc.dram_tensor("out", [P, NPS, NHID], odt, kind="ExternalOutput").ap()

    with tile.TileContext(nc) as tc, ExitStack() as ctx:
        cpool = ctx.enter_context(tc.tile_pool(name="consts", bufs=1))
        iota = cpool.tile([P, G, TC], DT.bfloat16)
        nc.sync.dma_start(iota[:], iota_h[:])

        sp = ctx.enter_context(tc.tile_pool(name="stream", bufs=2))
        mp = ctx.enter_context(tc.tile_pool(name="meta", bufs=2))
        ep = ctx.enter_context(tc.tile_pool(name="edge", bufs=2))
        mwp = ctx.enter_context(tc.tile_pool(name="mw", bufs=2))
        op = ctx.enter_context(tc.tile_pool(name="out", bufs=2))
        zp = ctx.enter_context(tc.tile_pool(name="z", bufs=8))
        pp = ctx.enter_context(tc.tile_pool(name="ps", bufs=8, space="PSUM"))

        def _evac(p):
            pl, pci = p
            outsb = op.tile([P, PPC, NHID], odt, tag="outsb")
            for k, ps in enumerate(pl):
                zinv = zp.tile([P, 1], DT.float32, tag="zinv")
                nc.vector.reciprocal(zinv[:], ps[:, NHID:NHID + 1])
                nc.scalar.activation(outsb[:, k, :], ps[:, 0:NHID],
                                     AF.Relu if relu else AF.Copy,
                                     scale=zinv[:])
            nc.sync.dma_start(out_h[:, pci * PPC:(pci + 1) * PPC, :], outsb[:])

        pend = None
        for ci in range(NCHUNK):
            t0 = ci * TC
            S = sp.tile([P, TC, CS], sdt, tag="S")
            nc.sync.dma_start(S[:], feats[:, t0:t0 + TC, :])
            pre = mp.tile([P, TC], DT.bfloat16, tag="pre")
            nc.sync.dma_start(pre[:], pre_h[:, t0:t0 + TC])
            dstl = mp.tile([P, TC], DT.bfloat16, tag="dstl")
            nc.sync.dma_start(dstl[:], dst_h[:, t0:t0 + TC])

            lk = ep.tile([P, TC], DT.float32, tag="lk")
            nc.vector.tensor_scalar(out=lk[:], in0=pre[:], scalar1=NEG_SLOPE,
                                    scalar2=None, op0=ALU.mult)
            nc.vector.tensor_tensor(out=lk[:], in0=lk[:], in1=pre[:], op=ALU.max)
            w = ep.tile([P, TC], DT.bfloat16, tag="w")
            nc.scalar.activation(w[:], lk[:], AF.Exp)

            M = mwp.tile([P, G, TC], DT.bfloat16, tag="M")
            nc.vector.tensor_tensor(
                out=M[:], in0=dstl[:, None, :].broadcast_to([P, G, TC]),
                in1=iota[:], op=ALU.is_equal)
            Mw = mwp.tile([P, G, TC], DT.bfloat16, tag="Mw")
            nc.vector.tensor_tensor(
                out=Mw[:], in0=M[:], in1=w[:, None, :].broadcast_to([P, G, TC]),
                op=ALU.mult)

            if pend is not None:
                _evac(pend)
            ps_list = []
            for k in range(PPC):
                ps = pp.tile([P, CS], DT.float32, tag="ps")
                for j in range(4):
                    gl = k * 4 + j
                    tt = gl * TPG
                    for t in range(TPG):
                        nc.tensor.matmul(ps[G * j:G * (j + 1), :],
                                         lhsT=Mw[:, :, tt + t],
                                         rhs=S[:, tt + t, :],
                                         start=(t == 0), stop=(t == TPG - 1),
                                         tile_position=(0, G * j))
                ps_list.append(ps)
            pend = (ps_list, ci)
        _evac(pend)
    nc.compile()
    return nc


def _get(key, builder, *a):
    if key not in _CACHE:
        _CACHE[key] = builder(*a)
    return _CACHE[key]


# ------------------------------------------------------------------ host ----

def _bin_pack(deg):
    """LPT: assign NSH nodes to NGRP bins of exactly G slots, load<=TPG*P.
    Returns perm [NGRP*G] int32 (node id or -1 for pad)."""
    import heapq
    order = np.argsort(-deg, kind="stable")
    heap = [(0, g) for g in range(NGRP)]
    heapq.heapify(heap)
    bins = [[] for _ in range(NGRP)]
    spill = []
    for n in order:
        d = int(deg[n])
        load, g = heapq.heappop(heap)
        while len(bins[g]) >= G:
            load, g = heapq.heappop(heap)
        bins[g].append(n)
        nl = load + d
        if nl > TPG * P:
            raise RuntimeError(f"bin overflow {nl}")
        if len(bins[g]) < G:
            heapq.heappush(heap, (nl, g))
        else:
            spill.append((nl, g))
    perm = np.full(NGRP * G, -1, dtype=np.int64)
    for g, lst in enumerate(bins):
        perm[g * G:g * G + len(lst)] = lst
    return perm


def _prep_graph(edge_index):
    """Per-core slot layout. Returns list of dicts."""
    ei = np.asarray(edge_index)
    src = np.concatenate([ei[0], np.arange(N, dtype=ei.dtype)]).astype(np.int64)
    dst = np.concatenate([ei[1], np.arange(N, dtype=ei.dtype)]).astype(np.int64)
    owner = dst // NSH
    cores = []
    for c in range(NCORES):
        sel = owner == c
        s_c = src[sel]
        d_c = dst[sel] - c * NSH                     # local dst 0..12499
        deg = np.bincount(d_c, minlength=NSH)
        perm = _bin_pack(deg)                        # [12544] node or -1
        # node -> (group, j)
        slot_of_node = np.full(NSH, -1, dtype=np.int64)
        valid = perm >= 0
        slot_of_node[perm[valid]] = np.nonzero(valid)[0]
        key = slot_of_node[d_c]                      # g*32+j per edge
        order = np.argsort(key, kind="stable")
        s_c, d_c, key = s_c[order], d_c[order], key[order]
        grp = key // G
        # position within group: running index
        gstart = np.searchsorted(grp, np.arange(NGRP))
        cnt = np.diff(np.append(gstart, len(grp)))
        if cnt.max() > TPG * P:
            raise RuntimeError(f"group overflow {cnt.max()}")
        pos = np.arange(len(grp)) - gstart[grp]
        slot = grp * (TPG * P) + pos                 # linear slot in [0, NSLOT)
        slot_src = np.zeros(NSLOT, dtype=np.int64)
        slot_dst_g = np.zeros(NSLOT, dtype=np.int64) # global dst per slot
        dstloc = np.zeros(NSLOT, dtype=np.float32)
        wkill = np.full(NSLOT, True)                 # pad slots
        slot_src[slot] = s_c
        slot_dst_g[slot] = d_c + c * NSH
        dstloc[slot] = key % G
        wkill[slot] = False
        cores.append(dict(slot_src=slot_src, slot_dst=slot_dst_g,
                          dstloc=dstloc.astype(BF16), wkill=wkill, perm=perm))
    return cores


def _make_iota():
    i = np.arange(G, dtype=np.float32)[None, :, None]
    return np.broadcast_to(i, (P, G, TC)).astype(BF16).copy()


def _feats_stream(table66, core):
    """table66 [N,65] (col 64 = 1.0). -> [P, NT, CS] stream dtype."""
    flat = table66[core["slot_src"]]                 # [NSLOT, 65]
    flat[core["wkill"], 64] = 0                      # ones col 0 on pad slots
    return np.ascontiguousarray(
        flat.reshape(NT, P, CS).transpose(1, 0, 2))


def _meta_streams(pre_f32, core):
    pre = pre_f32.copy()
    pre[core["wkill"]] = -30000.0
    pre = pre.astype(BF16).reshape(NT, P).T.copy()
    dstl = core["dstloc"].reshape(NT, P).T.copy()
    return pre, dstl


def _run_lin(nc_lin, xT_list, W, a_src, a_dst):
    Wb = np.ascontiguousarray(W, dtype=np.float32).astype(BF16)
    WTb = np.ascontiguousarray(W.T, dtype=np.float32).astype(BF16)
    ap = np.stack([a_src, a_dst], axis=1).astype(np.float32).astype(BF16)
    in_maps = [{"xT": xT_list[c], "w": Wb, "wT": WTb, "apair": ap}
               for c in range(NCORES)]
    res = run_bass_kernel_spmd(nc_lin, in_maps, core_ids=list(range(NCORES)))
    # assemble global tables: xs [N,64] bf16 (from cols 0:NSH), s,d [N] f32
    xs = np.empty((N, NHID + 2), dtype=np.float32)
    for c in range(NCORES):
        xs[c * NSH:(c + 1) * NSH] = \
            res.results[c]["xs_sd"][:, :NSH].T.astype(np.float32)
    return xs[:, 0:NHID], xs[:, NHID], xs[:, NHID + 1]


def _run_agg(nc_agg, cores, xs, s, d, iota):
    table66 = np.empty((N, CS), dtype=np.float32)
    table66[:, 0:NHID] = xs
    table66[:, NHID] = 1.0
    table66 = table66.astype(F8 if FP8 else BF16)
    in_maps = []
    for core in cores:
        pre = s[core["slot_src"]] + d[core["slot_dst"]]
        pre_st, dst_st = _meta_streams(pre, core)
        in_maps.append({"feats": _feats_stream(table66, core),
                        "pre": pre_st, "dstloc": dst_st, "iota": iota})
    res = run_bass_kernel_spmd(nc_agg, in_maps, core_ids=list(range(NCORES)))
    # out [P, NPS, 64] -> rows r = pstile*128+p = g*32+j -> node perm[g*32+j]
    full = np.empty((N, NHID), dtype=np.float32)
    for c, core in enumerate(cores):
        o = res.results[c]["out"]                   # [P, NPS, 64]
        rows = o.transpose(1, 0, 2).reshape(NODES_PAD, NHID).astype(np.float32)
        valid = core["perm"] >= 0
        full[c * NSH + core["perm"][valid]] = rows[valid]
    return full


def kernel(x, W1, att_src1, att_dst1, W2, att_src2, att_dst2, edge_index):
    x = np.asarray(x, dtype=np.float32)
    W1 = np.asarray(W1, dtype=np.float32)
    W2 = np.asarray(W2, dtype=np.float32)
    a_s1 = np.asarray(att_src1, dtype=np.float32)
    a_d1 = np.asarray(att_dst1, dtype=np.float32)
    a_s2 = np.asarray(att_src2, dtype=np.float32)
    a_d2 = np.asarray(att_dst2, dtype=np.float32)

    cores = _prep_graph(edge_index)
    iota = _make_iota()

    ncA = _get(("lin", NFEAT), _build_lin, NFEAT)
    ncB2 = _get(("lin", NHID), _build_lin, NHID)
    ncB = _get(("agg", True), _build_agg, True, False)
    ncC = _get(("agg", False), _build_agg, False, True)

    # layer 1
    xb = x.astype(BF16)
    xT_list = []
    for c in range(NCORES):
        xt = np.zeros((NFEAT, NODES_PAD), dtype=BF16)
        xt[:, :NSH] = xb[c * NSH:(c + 1) * NSH].T
        xT_list.append(xt)
    xs1, s1, d1 = _run_lin(ncA, xT_list, W1, a_s1, a_d1)
    h = _run_agg(ncB, cores, xs1, s1, d1, iota)

    # layer 2
    hb = h.astype(BF16)
    hT_list = []
    for c in range(NCORES):
        ht = np.zeros((NHID, NODES_PAD), dtype=BF16)
        ht[:, :NSH] = hb[c * NSH:(c + 1) * NSH].T
        hT_list.append(ht)
    xs2, s2, d2 = _run_lin(ncB2, hT_list, W2, a_s2, a_d2)
    out = _run_agg(ncC, cores, xs2, s2, d2, iota)
    return out.astype(np.float32)
